# revision 11
# baseline (speedup 1.0000x reference)
"""Correlation (cost volume) kernel for Trainium2, 8-core data parallel.

Math (matches the reference):
  x1 = proj(input1), x2 = proj(input2)  (1x1 conv = per-pixel channel matmul)
  x2p = zero-pad(x2, 4 on each spatial side)
  out[b, di*9+dj, i, j] = sum_f x1[b,f,i,j] * x2p[b,f,i+di,j+dj] / sqrt(128)

Key algebra: <W x1 + b, W x2 + b> = x1 . (M x2 + W^T b) + b . (W x2 + b)
with M = W^T W.  With the harness' b = 0 the second term vanishes, so the
kernel correlates RAW x1 against z2 = (M x2 + W^T b) / sqrt(128), which is
computed on the host (it is 3% of the FLOPs and the device program is
DMA-bound, so it is free there) and shipped in bf16 — the same byte count
as shipping x2.

Device strategy (per core, 4 batches each):
  - correlation as 2D-block matmuls: stationary = raw x1 pixel block
    [128c, 8x16 px] (all 128 PE columns used), moving = zero-padded z2
    window [128c, 16x24] = 384 columns -> PSUM [128px, 384].  Each moving
    column is shared by up to 81 (di,dj) outputs, so the correlation
    costs only 3 moving columns per pixel (vs 11.25 for row-banded
    strips).
  - PSUM -> SBUF bf16 copies (split ACT/DVE) into 36-block super-tiles.
  - band DMA ships, per partition-row group r (pixel row r of the 8x16
    block, partitions 16r..16r+15), only the 256-element window starting
    at column 24*r of each block: pixel (r,c) needs columns
    24*(r+di)+(c+dj), i.e. window-local 24*di+c+dj <= 215 < 256.  The
    window is a uniform free-offset per 16-partition slice (a legal AP),
    256 elements = 512 bytes = exactly the full-rate DMA descriptor
    minimum.  This cuts band traffic from 768B to 512B per pixel and
    makes the shipped data r-de-skewed; the residual in-row shift
    out[p,di,dj] = ship[p, 24*di + (p%16) + dj] is a pure numpy
    as_strided view on the host.

All matmul operands are bf16 (PSUM accumulates fp32).
"""
import math

import numpy as np
import ml_dtypes

import concourse.bass as bass
import concourse.bacc as bacc
import concourse.tile as tile
import concourse.mybir as mybir
from concourse.bass_utils import run_bass_kernel_spmd

B, C, H, W = 32, 128, 96, 96
NCORES = 8
BLOC = B // NCORES          # 4 batches per core
PATCH = 9
R = PATCH // 2              # 4
PW = W + 2 * R              # 104 (z2 is shipped column-padded)
PH = H + 2 * R              # 104 (rows padded on device by memset)
NPIX = H * W                # 9216
BH, BW = 8, 16              # correlation block: 8 x 16 pixels = 128
MH, MW = BH + 2 * R, BW + 2 * R  # 16 x 24 moving window
MCOLS = MH * MW             # 384 moving columns per block
NBI, NBJ = H // BH, W // BW  # 12 x 6 blocks per image
NGRP = 4                    # band super-tiles per batch
NBLK = (NBI // NGRP) * NBJ  # 36 blocks per super-tile
WIN = 256                   # shipped window per partition per block (512B)
PAD = 56                    # super-tile tail pad (window overrun headroom)
WT = NBLK * MCOLS + PAD     # super-tile width: 13880 elements
OUT_DT = mybir.dt.bfloat16  # band DMA dtype (fp32 PSUM rounded once)

_cache: dict = {}


def _build_program():
    nc = bacc.Bacc(target_bir_lowering=False)
    bf = mybir.dt.bfloat16
    f32 = mybir.dt.float32

    x1d = nc.declare_dram_parameter("x1", [BLOC, C, NPIX], bf, isOutput=False)
    z2d = nc.declare_dram_parameter("z2", [BLOC, C, H * PW], bf,
                                    isOutput=False)
    bandd = nc.declare_dram_parameter(
        "band", [BLOC, NGRP, BH, BW, NBLK, WIN], OUT_DT, isOutput=True
    )

    with tile.TileContext(nc) as tc:
        with (
            tc.tile_pool(name="imgs", bufs=2) as imgs,
            tc.tile_pool(name="feats", bufs=2) as feats,
            tc.tile_pool(name="bands", bufs=6) as bands,
            tc.tile_pool(name="bps", bufs=6, space="PSUM") as bps,
        ):
            ncopy = 0

            def copy(dst, src):
                # split PSUM->SBUF copy load between DVE (~47%) and ACT
                nonlocal ncopy
                ncopy += 1
                if ncopy % 15 < 7:
                    nc.vector.tensor_copy(dst, src)
                else:
                    nc.scalar.copy(dst, src)

            loaded = {}

            def load(b):
                # issue input DMAs ahead of the band DMAs in SP's in-order
                # queue, so band issues (which wait on copies) never block
                # the next batch's input transfers
                z2 = feats.tile([C, PH * PW], bf, tag="z2")
                # interior rows arrive column-padded from the host;
                # top/bottom pad rows are zeroed on-chip (contiguous spans)
                nc.sync.dma_start(out=z2[:, R * PW:(R + H) * PW],
                                  in_=z2d[b, :, :])
                nc.vector.memset(z2[:, 0:R * PW], 0.0)
                nc.vector.memset(z2[:, (R + H) * PW:PH * PW], 0.0)
                x1t = imgs.tile([C, NPIX], bf, tag="x1")
                nc.sync.dma_start(out=x1t[:, :], in_=x1d[b, :, :])
                loaded[b] = (z2, x1t)

            load(0)
            for b in range(BLOC):
                if b + 1 < BLOC:
                    load(b + 1)
                z2, x1t = loaded.pop(b)
                z2v = z2[:, :].rearrange("c (r w) -> c r w", w=PW)
                # x1 is host-pre-blocked: [c, bi, bj, 128] with p = 16*r + c
                x1v = x1t[:, :].rearrange("c (bi bj p) -> c bi bj p",
                                          bj=NBJ, p=BH * BW)

                for g in range(NGRP):
                    bt = bands.tile([C, WT], OUT_DT, tag="bt")
                    # window-overrun headroom columns must be readable
                    nc.vector.memset(bt[:, NBLK * MCOLS:WT], 0.0)
                    for s in range(NBI // NGRP):
                        bi = (NBI // NGRP) * g + s
                        for bj in range(NBJ):
                            q = NBJ * s + bj
                            pb = bps.tile([C, MCOLS], f32, tag="pb")
                            nc.tensor.matmul(
                                pb[:, :],
                                x1v[:, bi, bj, :],
                                z2v[:, BH * bi:BH * bi + MH,
                                    BW * bj:BW * bj + MW],
                                start=True, stop=True,
                            )
                            copy(bt[:, bass.ts(q, MCOLS)], pb[:, :])
                    base = bt[:, :]
                    for r in range(BH):
                        shear = bass.AP(
                            base.tensor,
                            base.offset + (16 * r) * WT + 24 * r,
                            [[WT, 16], [MCOLS, NBLK], [1, WIN]],
                        )
                        nc.sync.dma_start(out=bandd[b, g, r, :, :, :],
                                          in_=shear)

    nc.compile()
    return nc


def kernel(input1, input2, proj_w, proj_b):
    if "nc" not in _cache:
        _cache["nc"] = _build_program()
    nc = _cache["nc"]

    w64 = np.asarray(proj_w, dtype=np.float64)
    b64 = np.asarray(proj_b, dtype=np.float64)
    s = 1.0 / math.sqrt(C)
    m = ((w64.T @ w64) * s).astype(np.float32)
    b2 = (w64.T @ b64 * s).astype(np.float32)

    # host projection z2 = M x2 + W^T b (fp32 BLAS), column-padded, bf16
    x2 = np.asarray(input2, dtype=np.float32).reshape(B, C, NPIX)
    z2 = np.matmul(m[None], x2) + b2[None, :, None]          # [B, C, 9216]
    z2c = np.zeros((B, C, H, PW), dtype=ml_dtypes.bfloat16)
    z2c[:, :, :, R:R + W] = z2.reshape(B, C, H, W).astype(ml_dtypes.bfloat16)
    z2c = z2c.reshape(B, C, H * PW)

    # pre-block x1 so each 8x16 correlation block is a contiguous
    # single-free-dim stationary slice: [b, c, bi, bj, 16*r + c']
    x1b = (np.asarray(input1)
           .reshape(B, C, NBI, BH, NBJ, BW)
           .transpose(0, 1, 2, 4, 3, 5)
           .reshape(B, C, NPIX)
           .astype(ml_dtypes.bfloat16))

    in_maps = []
    for k in range(NCORES):
        sl = slice(BLOC * k, BLOC * (k + 1))
        in_maps.append({
            "x1": np.ascontiguousarray(x1b[sl]),
            "z2": np.ascontiguousarray(z2c[sl]),
        })

    res = run_bass_kernel_spmd(nc, in_maps, list(range(NCORES)))

    # host de-skew: out[b, di*9+dj, 8*(6g+s)+r, 16*bj+c]
    #             = band[b, g, r, c, 6*s+bj, 24*di + c + dj]
    outs = []
    for k in range(NCORES):
        band = np.asarray(res.results[k]["band"])
        v = band.reshape(BLOC, NGRP, BH, BW, NBI // NGRP, NBJ, WIN)
        st = v.strides
        sel = np.lib.stride_tricks.as_strided(
            v,
            shape=(BLOC, PATCH, PATCH, NGRP, NBI // NGRP, BH, NBJ, BW),
            strides=(st[0], MW * st[6], st[6], st[1], st[4], st[2],
                     st[5], st[3] + st[6]),
        )
        outs.append(sel.astype(np.float32).reshape(BLOC, PATCH * PATCH, H, W))
    out = np.concatenate(outs, axis=0)

    if np.any(b64 != 0.0):
        # general-bias correction: b . pad(W x2 + b) term (zero in harness)
        y2 = np.einsum("fc,bchw->bfhw", w64, np.asarray(input2, np.float64))
        t = (np.einsum("f,bfhw->bhw", b64, y2 + b64[None, :, None, None]) * s)
        tp = np.pad(t, ((0, 0), (R, R), (R, R)))
        for di in range(PATCH):
            for dj in range(PATCH):
                out[:, di * PATCH + dj] += tp[:, di:di + H, dj:dj + W].astype(
                    np.float32)
    return out


# revision 12
# speedup vs baseline: 1.1341x; 1.1341x over previous
"""Correlation (cost volume) kernel for Trainium2, 8-core data parallel.

Math (matches the reference):
  x1 = proj(input1), x2 = proj(input2)  (1x1 conv = per-pixel channel matmul)
  x2p = zero-pad(x2, 4 on each spatial side)
  out[b, di*9+dj, i, j] = sum_f x1[b,f,i,j] * x2p[b,f,i+di,j+dj] / sqrt(128)

Key algebra: <W x1 + b, W x2 + b> = x1 . (M x2 + W^T b) + b . (W x2 + b)
with M = W^T W.  With the harness' b = 0 the second term vanishes, so the
kernel correlates RAW x1 against z2 = (M x2 + W^T b) / sqrt(128), which is
computed on the host (it is 3% of the FLOPs and the device program is
DMA-bound, so it is free there) and shipped in bf16 — the same byte count
as shipping x2.

Device strategy (per core, 4 batches each):
  - correlation as 2D-block matmuls: stationary = raw x1 pixel block
    [128c, 8x16 px] (all 128 PE columns used), moving = zero-padded z2
    window [128c, 16x24] = 384 columns -> PSUM [128px, 384].  Each moving
    column is shared by up to 81 (di,dj) outputs, so the correlation
    costs only 3 moving columns per pixel (vs 11.25 for row-banded
    strips).
  - PSUM -> SBUF bf16 copies (split ACT/DVE) into 36-block super-tiles.
  - band DMA ships, per partition-row group r (pixel row r of the 8x16
    block, partitions 16r..16r+15), only the 256-element window starting
    at column 24*r of each block: pixel (r,c) needs columns
    24*(r+di)+(c+dj), i.e. window-local 24*di+c+dj <= 215 < 256.  The
    window is a uniform free-offset per 16-partition slice (a legal AP),
    256 elements = 512 bytes = exactly the full-rate DMA descriptor
    minimum.  This cuts band traffic from 768B to 512B per pixel and
    makes the shipped data r-de-skewed; the residual in-row shift
    out[p,di,dj] = ship[p, 24*di + (p%16) + dj] is a pure numpy
    as_strided view on the host.

All matmul operands are bf16 (PSUM accumulates fp32).
"""
import math

import numpy as np
import ml_dtypes

import concourse.bass as bass
import concourse.bacc as bacc
import concourse.tile as tile
import concourse.mybir as mybir
from concourse.bass_utils import run_bass_kernel_spmd

B, C, H, W = 32, 128, 96, 96
NCORES = 8
BLOC = B // NCORES          # 4 batches per core
PATCH = 9
R = PATCH // 2              # 4
PW = W + 2 * R              # 104 (z2 is shipped column-padded)
PH = H + 2 * R              # 104 (rows padded on device by memset)
NPIX = H * W                # 9216
BH, BW = 8, 16              # correlation block: 8 x 16 pixels = 128
MH, MW = BH + 2 * R, BW + 2 * R  # 16 x 24 moving window
MCOLS = MH * MW             # 384 moving columns per block
NBI, NBJ = H // BH, W // BW  # 12 x 6 blocks per image
NGRP = 3                    # band super-tiles per batch
NBLK = (NBI // NGRP) * NBJ  # 36 blocks per super-tile
WIN = 256                   # shipped window per partition per block (512B)
PAD = 56                    # super-tile tail pad (window overrun headroom)
WT = NBLK * MCOLS + PAD     # super-tile width: 13880 elements
OUT_DT = mybir.dt.bfloat16  # band DMA dtype (fp32 PSUM rounded once)

_cache: dict = {}


def _build_program():
    nc = bacc.Bacc(target_bir_lowering=False)
    bf = mybir.dt.bfloat16
    f32 = mybir.dt.float32

    x1d = nc.declare_dram_parameter("x1", [BLOC, C, NPIX], bf, isOutput=False)
    z2d = nc.declare_dram_parameter("z2", [BLOC, C, H * PW], bf,
                                    isOutput=False)
    bandd = nc.declare_dram_parameter(
        "band", [BLOC, NGRP, BH, BW, NBLK, WIN], OUT_DT, isOutput=True
    )

    with tile.TileContext(nc) as tc:
        with (
            tc.tile_pool(name="imgs", bufs=3) as imgs,
            tc.tile_pool(name="feats", bufs=3) as feats,
            tc.tile_pool(name="bands", bufs=4) as bands,
            tc.tile_pool(name="bps", bufs=6, space="PSUM") as bps,
        ):
            ncopy = 0

            def copy(dst, src):
                # split PSUM->SBUF copy load between DVE (~47%) and ACT
                nonlocal ncopy
                ncopy += 1
                if ncopy % 15 < 7:
                    nc.vector.tensor_copy(dst, src)
                else:
                    nc.scalar.copy(dst, src)

            loaded = {}

            def load(b):
                # issue input DMAs ahead of the band DMAs in SP's in-order
                # queue, so band issues (which wait on copies) never block
                # the next batch's input transfers
                z2 = feats.tile([C, PH * PW], bf, tag="z2")
                # interior rows arrive column-padded from the host;
                # top/bottom pad rows are zeroed on-chip (contiguous spans)
                nc.sync.dma_start(out=z2[:, R * PW:(R + H) * PW],
                                  in_=z2d[b, :, :])
                nc.vector.memset(z2[:, 0:R * PW], 0.0)
                nc.vector.memset(z2[:, (R + H) * PW:PH * PW], 0.0)
                x1t = imgs.tile([C, NPIX], bf, tag="x1")
                nc.sync.dma_start(out=x1t[:, :], in_=x1d[b, :, :])
                loaded[b] = (z2, x1t)

            load(0)
            load(1)
            for b in range(BLOC):
                if b + 2 < BLOC:
                    load(b + 2)
                z2, x1t = loaded.pop(b)
                z2v = z2[:, :].rearrange("c (r w) -> c r w", w=PW)
                # x1 is host-pre-blocked: [c, bi, bj, 128] with p = 16*r + c
                x1v = x1t[:, :].rearrange("c (bi bj p) -> c bi bj p",
                                          bj=NBJ, p=BH * BW)

                for g in range(NGRP):
                    bt = bands.tile([C, WT], OUT_DT, tag="bt")
                    # window-overrun headroom columns must be readable
                    nc.vector.memset(bt[:, NBLK * MCOLS:WT], 0.0)
                    for s in range(NBI // NGRP):
                        bi = (NBI // NGRP) * g + s
                        for bj in range(NBJ):
                            q = NBJ * s + bj
                            pb = bps.tile([C, MCOLS], f32, tag="pb")
                            nc.tensor.matmul(
                                pb[:, :],
                                x1v[:, bi, bj, :],
                                z2v[:, BH * bi:BH * bi + MH,
                                    BW * bj:BW * bj + MW],
                                start=True, stop=True,
                            )
                            copy(bt[:, bass.ts(q, MCOLS)], pb[:, :])
                    base = bt[:, :]
                    for r in range(BH):
                        shear = bass.AP(
                            base.tensor,
                            base.offset + (16 * r) * WT + 24 * r,
                            [[WT, 16], [MCOLS, NBLK], [1, WIN]],
                        )
                        nc.sync.dma_start(out=bandd[b, g, r, :, :, :],
                                          in_=shear)

    nc.compile()
    return nc


def kernel(input1, input2, proj_w, proj_b):
    if "nc" not in _cache:
        _cache["nc"] = _build_program()
    nc = _cache["nc"]

    w64 = np.asarray(proj_w, dtype=np.float64)
    b64 = np.asarray(proj_b, dtype=np.float64)
    s = 1.0 / math.sqrt(C)
    m = ((w64.T @ w64) * s).astype(np.float32)
    b2 = (w64.T @ b64 * s).astype(np.float32)

    # host projection z2 = M x2 + W^T b (fp32 BLAS), column-padded, bf16
    x2 = np.asarray(input2, dtype=np.float32).reshape(B, C, NPIX)
    z2 = np.matmul(m[None], x2) + b2[None, :, None]          # [B, C, 9216]
    z2c = np.zeros((B, C, H, PW), dtype=ml_dtypes.bfloat16)
    z2c[:, :, :, R:R + W] = z2.reshape(B, C, H, W).astype(ml_dtypes.bfloat16)
    z2c = z2c.reshape(B, C, H * PW)

    # pre-block x1 so each 8x16 correlation block is a contiguous
    # single-free-dim stationary slice: [b, c, bi, bj, 16*r + c']
    x1b = (np.asarray(input1)
           .reshape(B, C, NBI, BH, NBJ, BW)
           .transpose(0, 1, 2, 4, 3, 5)
           .reshape(B, C, NPIX)
           .astype(ml_dtypes.bfloat16))

    in_maps = []
    for k in range(NCORES):
        sl = slice(BLOC * k, BLOC * (k + 1))
        in_maps.append({
            "x1": np.ascontiguousarray(x1b[sl]),
            "z2": np.ascontiguousarray(z2c[sl]),
        })

    res = run_bass_kernel_spmd(nc, in_maps, list(range(NCORES)))

    # host de-skew: out[b, di*9+dj, 8*(6g+s)+r, 16*bj+c]
    #             = band[b, g, r, c, 6*s+bj, 24*di + c + dj]
    outs = []
    for k in range(NCORES):
        band = np.asarray(res.results[k]["band"])
        v = band.reshape(BLOC, NGRP, BH, BW, NBI // NGRP, NBJ, WIN)
        st = v.strides
        sel = np.lib.stride_tricks.as_strided(
            v,
            shape=(BLOC, PATCH, PATCH, NGRP, NBI // NGRP, BH, NBJ, BW),
            strides=(st[0], MW * st[6], st[6], st[1], st[4], st[2],
                     st[5], st[3] + st[6]),
        )
        outs.append(sel.astype(np.float32).reshape(BLOC, PATCH * PATCH, H, W))
    out = np.concatenate(outs, axis=0)

    if np.any(b64 != 0.0):
        # general-bias correction: b . pad(W x2 + b) term (zero in harness)
        y2 = np.einsum("fc,bchw->bfhw", w64, np.asarray(input2, np.float64))
        t = (np.einsum("f,bfhw->bhw", b64, y2 + b64[None, :, None, None]) * s)
        tp = np.pad(t, ((0, 0), (R, R), (R, R)))
        for di in range(PATCH):
            for dj in range(PATCH):
                out[:, di * PATCH + dj] += tp[:, di:di + H, dj:dj + W].astype(
                    np.float32)
    return out
